# revision 8
# baseline (speedup 1.0000x reference)
"""Trainium2 Bass kernel for nn_LocalExperts (MoE grouped FFN).

out[e] = relu(x[e] @ wi[e]) @ wo[e]   for e in 0..7

Expert-parallel over 8 NeuronCores: core e computes expert e's FFN.
Per-core work: x [8192, 512] f32, wi [512, 2048], wo [2048, 512]

v3: all-bf16 matmul pipeline, PE runs matmuls only.
- x fp32 -> SBUF (DMA), cast to bf16 on the Pool engine (otherwise idle),
  transposed to xT [d, m] by the DMA XBAR (dma_start(transpose=True),
  16x128 hardware transpose tiles) straight into SBUF. No PE transposes.
- GEMM1: hT[f, m] = wi_bf[d, f].T @ xT[d, m], 1024-wide moving free dim
  (one PSUM tile of 2 banks per fc), accumulated over 4 d-chunks.
- relu via ACT -> hT bf16 (halves GEMM2 stationary load traffic).
- GEMM2: out[m, d] = hT[f, m].T @ wo_bf[f, d], 16 f-chunk accumulation.
- Weights DMA'd fp32 in 512KB chunks, cast to bf16 on Pool; GEMM groups
  only depend on the chunk they read, so compute starts ~16us in.
bf16 end-to-end rel err (vs fp32 reference, max-err/absmax): ~2.8e-3.
"""

import numpy as np

import concourse.mybir as mybir
from concourse import bacc
from concourse.tile import TileContext
from concourse.bass_utils import run_bass_kernel_spmd

E, W, C, D, F = 8, 8, 1024, 512, 2048
P = 128
M_TOT = W * C            # 8192 rows per expert
M_TILE = 1024            # rows per m-tile (GEMM1 moving free dim)
N_MT = M_TOT // M_TILE   # 8
MS = M_TILE // P         # 8 m-subtiles of 128 rows
DC = D // P              # 4 d-chunks
FC = F // P              # 16 f-chunks

F32 = mybir.dt.float32
BF16 = mybir.dt.bfloat16


def _build_nc():
    nc = bacc.Bacc(None, target_bir_lowering=False)

    x = nc.dram_tensor("x", [M_TOT, D], F32, kind="ExternalInput")
    wi = nc.dram_tensor("wi", [D, F], F32, kind="ExternalInput")
    wo = nc.dram_tensor("wo", [F, D], F32, kind="ExternalInput")
    out = nc.dram_tensor("out", [M_TOT, D], F32, kind="ExternalOutput")

    x_v = x.rearrange("(mt ms p) d -> mt p ms d", p=P, ms=MS)
    out_v = out.rearrange("(mt ms p) d -> mt p ms d", p=P, ms=MS)
    wi_v = wi.rearrange("(dc p) f -> p dc f", p=P)
    wo_v = wo.rearrange("(fc p) d -> p fc d", p=P)

    with TileContext(nc) as tc:
        with (
            tc.tile_pool(name="const", bufs=1) as cpool,
            tc.tile_pool(name="xin", bufs=2) as xin_pool,
            tc.tile_pool(name="xbf", bufs=2) as xbf_pool,
            tc.tile_pool(name="xt", bufs=2) as xt_pool,
            tc.tile_pool(name="ht", bufs=2) as ht_pool,
            tc.tile_pool(name="stg", bufs=2) as stg_pool,
            tc.tile_pool(name="osb", bufs=4) as o_pool,
            tc.tile_pool(name="h_ps", bufs=3, space="PSUM") as h_psum,
            tc.tile_pool(name="o_ps", bufs=2, space="PSUM") as o_psum,
        ):
            wi_bf = cpool.tile([P, DC, F], BF16)
            wo_bf = cpool.tile([P, FC, D], BF16)

            def load_x(mt):
                # DMA fp32 x in per-ms chunks; Pool casts each to bf16 and
                # the DMA XBAR transposes bf16 [m, d] -> xt [d-part, m].
                x_nat = xin_pool.tile([P, MS, D], F32)
                x_bf = xbf_pool.tile([P, MS, D], BF16)
                xt = xt_pool.tile([P, DC, M_TILE], BF16)
                for ms in range(MS):
                    nc.sync.dma_start(x_nat[:, ms], x_v[mt, :, ms])
                    nc.gpsimd.tensor_copy(x_bf[:, ms], x_nat[:, ms])
                    nc.sync.dma_start(
                        xt[:, :, ms * P : (ms + 1) * P],
                        x_bf[:, ms],
                        transpose=True,
                    )
                return xt

            xt = load_x(0)

            # Weights: fp32 DMA chunks -> staging, Pool cast -> bf16.
            for q in range(8):
                s = slice(q * (F // 8), (q + 1) * (F // 8))
                st = stg_pool.tile([P, DC, F // 8], F32, tag="stg")
                nc.sync.dma_start(st, wi_v[:, :, s])
                nc.gpsimd.tensor_copy(wi_bf[:, :, s], st)
            for q in range(8):
                s = slice(q * (FC // 8), (q + 1) * (FC // 8))
                st = stg_pool.tile([P, FC // 8, D], F32, tag="stg")
                nc.sync.dma_start(st, wo_v[:, s])
                nc.gpsimd.tensor_copy(wo_bf[:, s], st)

            def gemm1(xt):
                # hT[f, m]: per fc one 2-bank PSUM tile; a matmul output
                # must fit one PSUM bank (512 fp32), so the 1024 m-columns
                # are written as two 512-wide halves, drained by one ACT
                # relu -> bf16.
                HB = M_TILE // 2
                hT = ht_pool.tile([P, FC, M_TILE], BF16)
                for fc in range(FC):
                    hp = h_psum.tile([P, 2, HB], F32)
                    for half in range(2):
                        for dc in range(DC):
                            nc.tensor.matmul(
                                hp[:, half],
                                wi_bf[:, dc, fc * P : (fc + 1) * P],
                                xt[:, dc, half * HB : (half + 1) * HB],
                                start=(dc == 0),
                                stop=(dc == DC - 1),
                            )
                    nc.scalar.activation(
                        hT[:, fc, :],
                        hp,
                        mybir.ActivationFunctionType.Relu,
                    )
                return hT

            def gemm2(mt, hT):
                # out[m, d] per 128-row subtile
                for ms in range(MS):
                    op = o_psum.tile([P, D], F32)
                    for fc in range(FC):
                        nc.tensor.matmul(
                            op,
                            hT[:, fc, ms * P : (ms + 1) * P],
                            wo_bf[:, fc, :],
                            start=(fc == 0),
                            stop=(fc == FC - 1),
                        )
                    o_t = o_pool.tile([P, D], F32)
                    nc.vector.tensor_copy(o_t, op)
                    nc.sync.dma_start(out_v[mt, :, ms, :], o_t)

            for mt in range(N_MT):
                hT = gemm1(xt)
                if mt + 1 < N_MT:
                    xt = load_x(mt + 1)
                gemm2(mt, hT)

    nc.finalize()
    return nc


_CACHE = {}


def _get_nc():
    if "nc" not in _CACHE:
        _CACHE["nc"] = _build_nc()
    return _CACHE["nc"]


def _run(x, wi, wo, **spmd_kwargs):
    """x [E, 8192, 512], wi [E, 512, 2048], wo [E, 2048, 512] -> results."""
    nc = _get_nc()
    in_maps = [
        {
            "x": np.ascontiguousarray(x[e]),
            "wi": np.ascontiguousarray(wi[e]),
            "wo": np.ascontiguousarray(wo[e]),
        }
        for e in range(E)
    ]
    return nc, run_bass_kernel_spmd(nc, in_maps, core_ids=list(range(E)), **spmd_kwargs)


def kernel(dispatched_hidden_states, experts_capacity_usage=None, wi=None, wo=None):
    x = np.asarray(dispatched_hidden_states, dtype=np.float32).reshape(E, M_TOT, D)
    wi_ = np.asarray(wi, dtype=np.float32)
    wo_ = np.asarray(wo, dtype=np.float32)
    _, res = _run(x, wi_, wo_)
    out = np.stack([res.results[e]["out"] for e in range(E)])
    return out.reshape(E, W, C, D)
